# revision 12
# baseline (speedup 1.0000x reference)
"""Trainium2 Bass kernel for nn_CorrTorch_unfold (B=1, C=32, D=32, H=W=128).

Math (flat-remap unfold, see reference docstring): per depth slice d
  out[k2, h2, w2] = lrelu( sum_c x[c,d,h2,w2] * y_pad[c', d, h'+kh, w'+kw+c] )
  with n = 9c'+k' (k'=(kh,kw)), (k2, m) = divmod(n, 32),
  h2 = 4m+t, w2 = 4wb+j, partition p = h' = 32t+wb.

v2 design (all fp16, everything on DVE at the 2x_1p rate):
  - GpSimd (Pool) is NOT used: concurrent Pool tensor_tensor degrades DVE
    ~2-4x via the shared SBUF port pair (measured), a large net loss.
  - A[p, kh*4160 + c'*130 + w] = y_pad[c', d, p+kh, w]   (3 row-shifted DMAs)
  - XT64[p, s*128 + j*32 + c] = x[c, d, 4*(s%32)+t, 4wb+j], 64 m-slots
    (two copies of the 32 m-slots) so each 4-unit product group reads a
    fully contiguous 4608-elem slab: group k covers slots 4k .. 4k+35.
  - products: per group of 4 c'-units one TT mult [128, 4608]
    (in0 A 4-dim strided AP, in1/out contiguous, all even offsets -> 2x).
  - tree: chunks of 16 units (g=576), strided pairwise adds, all even
    offsets (2x); last level via "plane-split" T4 (1x write) so the final
    add reads two contiguous planes instead of an odd-offset stride-2 AP.
  - lrelu = 0.6*OS + 0.4*|OS|: two ACT passes + one DVE add (ACT has its
    own SBUF ports; never contends).

Sharding: D=32 depth slices, 4 per core across 8 cores. Host packs/unpacks
(pure permutations); device output is OS-packed [d, 128, 1152] fp16.
"""
import numpy as np

_PROG_CACHE = {}
_RUN_OPTS = {"trace": False}
_LAST_RESULT = {}

D_LOC = 4
N_CORES = 8
C = 32
H = W = 128
WPAD = 130
ABLK = C * WPAD          # 4160 elems per kh block of A
NSLOT = 64               # XT m-slots (two copies of 32)


def _build_program():
    import concourse.bass as bass
    import concourse.bacc as bacc
    import concourse.mybir as mybir
    from concourse.tile import TileContext
    from bass_rust import VecI64Pair

    f16 = mybir.dt.float16
    f32 = mybir.dt.float32
    mult = mybir.AluOpType.mult
    add = mybir.AluOpType.add

    def apv(base_ap, offset, dims):
        a = base_ap.copy()
        part = list(a.ap[0])
        a.ap = VecI64Pair([part] + [list(d) for d in dims])
        a.offset = a.offset + offset
        return a

    nc = bacc.Bacc()
    xt_in = nc.dram_tensor("xtin", [D_LOC, 128, NSLOT * 128], f16,
                           kind="ExternalInput")
    y_in = nc.dram_tensor("yin", [D_LOC, WPAD, C, WPAD], f16,
                          kind="ExternalInput")
    id_in = nc.dram_tensor("ident", [128, 128], f16, kind="ExternalInput")
    out = nc.dram_tensor("out", [D_LOC, 128, 1152], f16,
                         kind="ExternalOutput")

    with TileContext(nc) as tc:
        with tc.tile_pool(name="a", bufs=2) as apool, \
             tc.tile_pool(name="xt", bufs=2) as xtpool, \
             tc.tile_pool(name="pr", bufs=1) as prpool, \
             tc.tile_pool(name="tr", bufs=2) as trpool, \
             tc.tile_pool(name="id", bufs=1) as idpool, \
             tc.psum_pool(name="ps", bufs=4) as pspool, \
             tc.tile_pool(name="ab", bufs=2) as abpool, \
             tc.tile_pool(name="ot", bufs=4) as otpool:

            ID = idpool.tile([128, 128], f16)
            nc.sync.dma_start(ID[:], id_in[:])

            for d in range(D_LOC):
                # ---- loads ----
                # separate tiles per kh block / XT half so the tile
                # dependency tracker lets kh=0 chunk-0 products start as
                # soon as XTa + A0 land (deps are whole-tile).
                # XTa = slots [0,48) for chunk 0; XTb = slots [16,64)
                # (overlapping) for chunk 1.
                XTa = xtpool.tile([128, 48 * 128], f16, tag="xta")
                nc.sync.dma_start(XTa[:], xt_in[d, :, :48 * 128])
                Akh = []
                for kh in range(3):
                    Ak = apool.tile([128, ABLK], f16, tag=f"a{kh}")
                    src = y_in[:].copy()
                    src.ap = VecI64Pair([[ABLK, 128], [1, ABLK]])
                    src.offset = (d * WPAD + kh) * ABLK
                    nc.sync.dma_start(Ak[:], src)
                    Akh.append(Ak)
                XTb = xtpool.tile([128, 48 * 128], f16, tag="xtb")
                nc.sync.dma_start(XTb[:], xt_in[d, :, 16 * 128:])

                # ---- 2 chunks of 16 c'-units ----
                for ch in range(2):
                    c0 = 16 * ch
                    P = prpool.tile([128, 16 * 1152], f16, tag="p")
                    # products: 4 groups of 4 units x 3 kh, one TT each
                    # (TensorTensor ISA mem patterns allow at most 3 free
                    # dims, so the kh axis gets its own instruction)
                    XTc = XTa if ch == 0 else XTb
                    sbase = 0 if ch == 0 else 16
                    for kh in range(3):
                        for gi in range(4):
                            kk = 4 * ch + gi        # global group id 0..7
                            cp0 = 4 * kk            # first c' of group
                            s0 = 4 * kk             # first XT slot of group
                            in0 = apv(Akh[kh][:], cp0 * WPAD,
                                      [[WPAD, 4], [1, 3], [1, 128]])
                            in1 = apv(XTc[:], (s0 - sbase + 3 * kh) * 128,
                                      [[1152, 4], [1, 384]])
                            po = apv(P[:], gi * 4608 + kh * 384,
                                     [[1152, 4], [1, 384]])
                            nc.vector.tensor_tensor(po, in0, in1, mult)
                    # tree over c2 (innermost 32), g = 16*36 = 576 groups
                    g = 576
                    T1 = trpool.tile([128, g * 16], f16, tag="t1")
                    nc.vector.tensor_tensor(
                        apv(T1[:], 0, [[1, g * 16]]),
                        apv(P[:], 0, [[32, g], [1, 16]]),
                        apv(P[:], 16, [[32, g], [1, 16]]), add)
                    # Remaining 16-way sum on the PE: identity-stationary
                    # matmuls accumulating the 16 strided T1 views into a
                    # PSUM fp32 bank (2 groups of 288 columns per chunk),
                    # then lrelu straight off PSUM (ACT) + one DVE add.
                    for h in range(2):
                        PS = pspool.tile([128, 288], f32, tag="ps")
                        for j in range(16):
                            nc.tensor.matmul(
                                PS[:], ID[:],
                                apv(T1[:], 288 * 16 * h + j, [[16, 288]]),
                                start=(j == 0), stop=(j == 15))
                        AB = abpool.tile([128, 288], f16, tag="ab")
                        CC = abpool.tile([128, 288], f16, tag="cc")
                        nc.scalar.activation(
                            AB[:], PS[:], mybir.ActivationFunctionType.Abs,
                            scale=0.4)
                        nc.scalar.activation(
                            CC[:], PS[:], mybir.ActivationFunctionType.Copy,
                            scale=0.6)
                        OT = otpool.tile([128, 288], f16, tag="ot")
                        nc.vector.tensor_tensor(OT[:], CC[:], AB[:], add)
                        nc.sync.dma_start(
                            out[d, :, c0 * 36 + 288 * h:
                                c0 * 36 + 288 * h + 288], OT[:])

    nc.finalize()
    return nc


def _get_program():
    if "nc" not in _PROG_CACHE:
        _PROG_CACHE["nc"] = _build_program()
    return _PROG_CACHE["nc"]


def _pack_xt(x):  # x [1,32,32,128,128] f32 -> [32, 128, NSLOT*128] fp16
    B, C_, D, H_, W_ = x.shape
    xt = np.zeros((D, 128, NSLOT, 128), np.float32)
    xd = x[0]  # [C, D, H, W]
    s = np.arange(NSLOT)
    m = s % 32
    for t in range(4):
        v = xd[:, :, 4 * m + t, :].reshape(C_, D, NSLOT, 32, 4)  # c d s wb j
        xt[:, 32 * t:32 * t + 32, :, :] = (
            v.transpose(1, 3, 2, 4, 0).reshape(D, 32, NSLOT, 128))
    return np.ascontiguousarray(xt.reshape(D, 128, NSLOT * 128)
                                ).astype(np.float16)


def kernel(x: np.ndarray, y: np.ndarray) -> np.ndarray:
    from concourse.bass_utils import run_bass_kernel_spmd

    x = np.ascontiguousarray(np.asarray(x, dtype=np.float32))
    y = np.ascontiguousarray(np.asarray(y, dtype=np.float32))
    B, C_, D, H_, W_ = x.shape
    assert (B, C_, D, H_, W_) == (1, 32, 32, 128, 128)

    # host prep: depth-shifted, H/W-padded y (fp16); packed XT slabs
    y_sp = np.zeros((D, WPAD, C_, WPAD), np.float16)
    y_sp[1:, 1:129, :, 1:129] = y[0].transpose(1, 2, 0, 3)[:-1].astype(
        np.float16)
    xt = _pack_xt(x)

    nc = _get_program()
    ident = np.eye(128, dtype=np.float16)
    in_maps = [
        {"xtin": xt[4 * j:4 * j + 4], "yin": y_sp[4 * j:4 * j + 4],
         "ident": ident}
        for j in range(N_CORES)
    ]
    res = run_bass_kernel_spmd(nc, in_maps, core_ids=list(range(N_CORES)),
                               trace=_RUN_OPTS["trace"])
    _LAST_RESULT["res"] = res
    packed = np.concatenate(
        [np.asarray(res.results[j]["out"], np.float32)
         for j in range(N_CORES)], axis=0)  # [32, 128, 1152]

    # host unpermute: [d, p, c'*36 + k*4 + j] -> [1, 9, D, H, W]
    a = packed.reshape(D, 4, 32, 32, 9, 4)                 # d t wb c' k j
    a = a.transpose(3, 4, 0, 1, 2, 5)                      # c' k d t wb j
    a = np.ascontiguousarray(a).reshape(9, 32, D, 4, 32, 4)  # k2 m d t wb j
    a = a.transpose(0, 2, 1, 3, 4, 5)                      # k2 d m t wb j
    a = np.ascontiguousarray(a).reshape(9, D, 128, 128)
    return a[None].astype(np.float32)


# revision 13
# speedup vs baseline: 1.4110x; 1.4110x over previous
"""Trainium2 Bass kernel for nn_CorrTorch_unfold (B=1, C=32, D=32, H=W=128).

Math (flat-remap unfold, see reference docstring): per depth slice d
  out[k2, h2, w2] = lrelu( sum_c x[c,d,h2,w2] * y_pad[c', d, h'+kh, w'+kw+c] )
  with n = 9c'+k' (k'=(kh,kw)), (k2, m) = divmod(n, 32),
  h2 = 4m+t, w2 = 4wb+j, partition p = h' = 32t+wb.

v2 design (all fp16, everything on DVE at the 2x_1p rate):
  - GpSimd (Pool) is NOT used: concurrent Pool tensor_tensor degrades DVE
    ~2-4x via the shared SBUF port pair (measured), a large net loss.
  - A[p, kh*4160 + c'*130 + w] = y_pad[c', d, p+kh, w]   (3 row-shifted DMAs)
  - XT64[p, s*128 + j*32 + c] = x[c, d, 4*(s%32)+t, 4wb+j], 64 m-slots
    (two copies of the 32 m-slots) so each 4-unit product group reads a
    fully contiguous 4608-elem slab: group k covers slots 4k .. 4k+35.
  - products: per group of 4 c'-units one TT mult [128, 4608]
    (in0 A 4-dim strided AP, in1/out contiguous, all even offsets -> 2x).
  - tree: chunks of 16 units (g=576), strided pairwise adds, all even
    offsets (2x); last level via "plane-split" T4 (1x write) so the final
    add reads two contiguous planes instead of an odd-offset stride-2 AP.
  - lrelu = 0.6*OS + 0.4*|OS|: two ACT passes + one DVE add (ACT has its
    own SBUF ports; never contends).

Sharding: D=32 depth slices, 4 per core across 8 cores. Host packs/unpacks
(pure permutations); device output is OS-packed [d, 128, 1152] fp16.
"""
import numpy as np

_PROG_CACHE = {}
_RUN_OPTS = {"trace": False}
_LAST_RESULT = {}

D_LOC = 4
N_CORES = 8
C = 32
H = W = 128
WPAD = 130
ABLK = C * WPAD          # 4160 elems per kh block of A
NSLOT = 64               # XT m-slots (two copies of 32)


def _build_program():
    import concourse.bass as bass
    import concourse.bacc as bacc
    import concourse.mybir as mybir
    from concourse.tile import TileContext
    from bass_rust import VecI64Pair

    f16 = mybir.dt.float16
    f32 = mybir.dt.float32
    mult = mybir.AluOpType.mult
    add = mybir.AluOpType.add

    def apv(base_ap, offset, dims):
        a = base_ap.copy()
        part = list(a.ap[0])
        a.ap = VecI64Pair([part] + [list(d) for d in dims])
        a.offset = a.offset + offset
        return a

    nc = bacc.Bacc()
    xt_in = nc.dram_tensor("xtin", [D_LOC, 128, NSLOT * 128], f16,
                           kind="ExternalInput")
    y_in = nc.dram_tensor("yin", [D_LOC, WPAD, C, WPAD], f16,
                          kind="ExternalInput")
    id_in = nc.dram_tensor("ident", [128, 128], f16, kind="ExternalInput")
    out = nc.dram_tensor("out", [D_LOC, 128, 1152], f16,
                         kind="ExternalOutput")

    with TileContext(nc) as tc:
        with tc.tile_pool(name="a", bufs=2) as apool, \
             tc.tile_pool(name="xt", bufs=2) as xtpool, \
             tc.tile_pool(name="pr", bufs=1) as prpool, \
             tc.tile_pool(name="tr", bufs=2) as trpool, \
             tc.tile_pool(name="id", bufs=1) as idpool, \
             tc.psum_pool(name="ps", bufs=4) as pspool, \
             tc.tile_pool(name="ab", bufs=2) as abpool, \
             tc.tile_pool(name="ot", bufs=4) as otpool:

            ID = idpool.tile([128, 128], f16)
            nc.sync.dma_start(ID[:], id_in[:])

            pending = []   # deferred (AB, CC, dram_slice) final adds

            def flush_pending():
                for ab, cc, dst in pending:
                    OT = otpool.tile([128, 288], f16, tag="ot")
                    nc.vector.tensor_tensor(OT[:], cc[:], ab[:], add)
                    nc.sync.dma_start(dst, OT[:])
                pending.clear()

            for d in range(D_LOC):
                # ---- loads ----
                # separate tiles per kh block / XT half so the tile
                # dependency tracker lets kh=0 chunk-0 products start as
                # soon as XTa + A0 land (deps are whole-tile).
                # XTa = slots [0,48) for chunk 0; XTb = slots [16,64)
                # (overlapping) for chunk 1.
                XTa = xtpool.tile([128, 48 * 128], f16, tag="xta")
                nc.sync.dma_start(XTa[:], xt_in[d, :, :48 * 128])
                Akh = []
                for kh in range(3):
                    Ak = apool.tile([128, ABLK], f16, tag=f"a{kh}")
                    src = y_in[:].copy()
                    src.ap = VecI64Pair([[ABLK, 128], [1, ABLK]])
                    src.offset = (d * WPAD + kh) * ABLK
                    nc.sync.dma_start(Ak[:], src)
                    Akh.append(Ak)
                XTb = xtpool.tile([128, 48 * 128], f16, tag="xtb")
                nc.sync.dma_start(XTb[:], xt_in[d, :, 16 * 128:])

                # ---- 2 chunks of 16 c'-units ----
                for ch in range(2):
                    c0 = 16 * ch
                    P = prpool.tile([128, 16 * 1152], f16, tag="p")
                    # products: 4 groups of 4 units x 3 kh, one TT each
                    # (TensorTensor ISA mem patterns allow at most 3 free
                    # dims, so the kh axis gets its own instruction)
                    XTc = XTa if ch == 0 else XTb
                    sbase = 0 if ch == 0 else 16
                    for kh in range(3):
                        for gi in range(4):
                            kk = 4 * ch + gi        # global group id 0..7
                            cp0 = 4 * kk            # first c' of group
                            s0 = 4 * kk             # first XT slot of group
                            in0 = apv(Akh[kh][:], cp0 * WPAD,
                                      [[WPAD, 4], [1, 3], [1, 128]])
                            in1 = apv(XTc[:], (s0 - sbase + 3 * kh) * 128,
                                      [[1152, 4], [1, 384]])
                            po = apv(P[:], gi * 4608 + kh * 384,
                                     [[1152, 4], [1, 384]])
                            nc.vector.tensor_tensor(po, in0, in1, mult)
                    # tree over c2 (innermost 32), g = 16*36 = 576 groups
                    g = 576
                    T1 = trpool.tile([128, g * 16], f16, tag="t1")
                    nc.vector.tensor_tensor(
                        apv(T1[:], 0, [[1, g * 16]]),
                        apv(P[:], 0, [[32, g], [1, 16]]),
                        apv(P[:], 16, [[32, g], [1, 16]]), add)
                    # Remaining 16-way sum on the PE: identity-stationary
                    # matmuls accumulating the 16 strided T1 views into a
                    # PSUM fp32 bank (2 groups of 288 columns per chunk),
                    # then lrelu straight off PSUM (ACT) + one DVE add.
                    # the DVE final adds for the PREVIOUS chunk are only
                    # emitted now, so PE/ACT have a whole chunk of slack
                    # before DVE needs their results (no DVE stall).
                    flush_pending()
                    for h in range(2):
                        PS = pspool.tile([128, 288], f32, tag="ps")
                        for j in range(16):
                            nc.tensor.matmul(
                                PS[:], ID[:],
                                apv(T1[:], 288 * 16 * h + j, [[16, 288]]),
                                start=(j == 0), stop=(j == 15))
                        AB = abpool.tile([128, 288], f16, tag=f"ab{h}")
                        CC = abpool.tile([128, 288], f16, tag=f"cc{h}")
                        nc.scalar.activation(
                            AB[:], PS[:], mybir.ActivationFunctionType.Abs,
                            scale=0.4)
                        nc.scalar.activation(
                            CC[:], PS[:], mybir.ActivationFunctionType.Copy,
                            scale=0.6)
                        pending.append(
                            (AB, CC,
                             out[d, :, c0 * 36 + 288 * h:
                                 c0 * 36 + 288 * h + 288]))

            flush_pending()
    nc.finalize()
    return nc


def _get_program():
    if "nc" not in _PROG_CACHE:
        _PROG_CACHE["nc"] = _build_program()
    return _PROG_CACHE["nc"]


def _pack_xt(x):  # x [1,32,32,128,128] f32 -> [32, 128, NSLOT*128] fp16
    B, C_, D, H_, W_ = x.shape
    xt = np.zeros((D, 128, NSLOT, 128), np.float32)
    xd = x[0]  # [C, D, H, W]
    s = np.arange(NSLOT)
    m = s % 32
    for t in range(4):
        v = xd[:, :, 4 * m + t, :].reshape(C_, D, NSLOT, 32, 4)  # c d s wb j
        xt[:, 32 * t:32 * t + 32, :, :] = (
            v.transpose(1, 3, 2, 4, 0).reshape(D, 32, NSLOT, 128))
    return np.ascontiguousarray(xt.reshape(D, 128, NSLOT * 128)
                                ).astype(np.float16)


def kernel(x: np.ndarray, y: np.ndarray) -> np.ndarray:
    from concourse.bass_utils import run_bass_kernel_spmd

    x = np.ascontiguousarray(np.asarray(x, dtype=np.float32))
    y = np.ascontiguousarray(np.asarray(y, dtype=np.float32))
    B, C_, D, H_, W_ = x.shape
    assert (B, C_, D, H_, W_) == (1, 32, 32, 128, 128)

    # host prep: depth-shifted, H/W-padded y (fp16); packed XT slabs
    y_sp = np.zeros((D, WPAD, C_, WPAD), np.float16)
    y_sp[1:, 1:129, :, 1:129] = y[0].transpose(1, 2, 0, 3)[:-1].astype(
        np.float16)
    xt = _pack_xt(x)

    nc = _get_program()
    ident = np.eye(128, dtype=np.float16)
    in_maps = [
        {"xtin": xt[4 * j:4 * j + 4], "yin": y_sp[4 * j:4 * j + 4],
         "ident": ident}
        for j in range(N_CORES)
    ]
    res = run_bass_kernel_spmd(nc, in_maps, core_ids=list(range(N_CORES)),
                               trace=_RUN_OPTS["trace"])
    _LAST_RESULT["res"] = res
    packed = np.concatenate(
        [np.asarray(res.results[j]["out"], np.float32)
         for j in range(N_CORES)], axis=0)  # [32, 128, 1152]

    # host unpermute: [d, p, c'*36 + k*4 + j] -> [1, 9, D, H, W]
    a = packed.reshape(D, 4, 32, 32, 9, 4)                 # d t wb c' k j
    a = a.transpose(3, 4, 0, 1, 2, 5)                      # c' k d t wb j
    a = np.ascontiguousarray(a).reshape(9, 32, D, 4, 32, 4)  # k2 m d t wb j
    a = a.transpose(0, 2, 1, 3, 4, 5)                      # k2 d m t wb j
    a = np.ascontiguousarray(a).reshape(9, D, 128, 128)
    return a[None].astype(np.float32)


# revision 15
# speedup vs baseline: 1.4239x; 1.0092x over previous
"""Trainium2 Bass kernel for nn_CorrTorch_unfold (B=1, C=32, D=32, H=W=128).

Math (flat-remap unfold, see reference docstring): per depth slice d
  out[k2, h2, w2] = lrelu( sum_c x[c,d,h2,w2] * y_pad[c', d, h'+kh, w'+kw+c] )
  with n = 9c'+k' (k'=(kh,kw)), (k2, m) = divmod(n, 32),
  h2 = 4m+t, w2 = 4wb+j, partition p = h' = 32t+wb.

v2 design (all fp16, everything on DVE at the 2x_1p rate):
  - GpSimd (Pool) is NOT used: concurrent Pool tensor_tensor degrades DVE
    ~2-4x via the shared SBUF port pair (measured), a large net loss.
  - A[p, kh*4160 + c'*130 + w] = y_pad[c', d, p+kh, w]   (3 row-shifted DMAs)
  - XT64[p, s*128 + j*32 + c] = x[c, d, 4*(s%32)+t, 4wb+j], 64 m-slots
    (two copies of the 32 m-slots) so each 4-unit product group reads a
    fully contiguous 4608-elem slab: group k covers slots 4k .. 4k+35.
  - products: per group of 4 c'-units one TT mult [128, 4608]
    (in0 A 4-dim strided AP, in1/out contiguous, all even offsets -> 2x).
  - tree: chunks of 16 units (g=576), strided pairwise adds, all even
    offsets (2x); last level via "plane-split" T4 (1x write) so the final
    add reads two contiguous planes instead of an odd-offset stride-2 AP.
  - lrelu = 0.6*OS + 0.4*|OS|: two ACT passes + one DVE add (ACT has its
    own SBUF ports; never contends).

Sharding: D=32 depth slices, 4 per core across 8 cores. Host packs/unpacks
(pure permutations); device output is OS-packed [d, 128, 1152] fp16.
"""
import numpy as np

_PROG_CACHE = {}
_RUN_OPTS = {"trace": False}
_LAST_RESULT = {}

D_LOC = 4
N_CORES = 8
C = 32
H = W = 128
WPAD = 130
ABLK = C * WPAD          # 4160 elems per kh block of A
NSLOT = 64               # XT m-slots (two copies of 32)


def _build_program():
    import concourse.bass as bass
    import concourse.bacc as bacc
    import concourse.mybir as mybir
    from concourse.tile import TileContext
    from bass_rust import VecI64Pair

    f16 = mybir.dt.float16
    f32 = mybir.dt.float32
    mult = mybir.AluOpType.mult
    add = mybir.AluOpType.add

    def apv(base_ap, offset, dims):
        a = base_ap.copy()
        part = list(a.ap[0])
        a.ap = VecI64Pair([part] + [list(d) for d in dims])
        a.offset = a.offset + offset
        return a

    nc = bacc.Bacc()
    xt_in = nc.dram_tensor("xtin", [D_LOC, 128, NSLOT * 128], f16,
                           kind="ExternalInput")
    y_in = nc.dram_tensor("yin", [D_LOC, WPAD, C, WPAD], f16,
                          kind="ExternalInput")
    id_in = nc.dram_tensor("ident", [128, 128], f16, kind="ExternalInput")
    out = nc.dram_tensor("out", [D_LOC, 128, 1152], f16,
                         kind="ExternalOutput")

    with TileContext(nc) as tc:
        with tc.tile_pool(name="a", bufs=2) as apool, \
             tc.tile_pool(name="xt", bufs=2) as xtpool, \
             tc.tile_pool(name="pr", bufs=1) as prpool, \
             tc.tile_pool(name="tr", bufs=3) as trpool, \
             tc.tile_pool(name="id", bufs=1) as idpool, \
             tc.psum_pool(name="ps", bufs=6) as pspool, \
             tc.tile_pool(name="ab", bufs=3) as abpool, \
             tc.tile_pool(name="ot", bufs=4) as otpool:

            ID = idpool.tile([128, 128], f16)
            nc.sync.dma_start(ID[:], id_in[:])

            pending = []   # deferred (AB, CC, dram_slice) final adds

            def flush_pending(keep=0):
                while len(pending) > keep:
                    ab, cc, dst = pending.pop(0)
                    OT = otpool.tile([128, 288], f16, tag="ot")
                    nc.vector.tensor_tensor(OT[:], cc[:], ab[:], add)
                    nc.sync.dma_start(dst, OT[:])

            for d in range(D_LOC):
                # ---- loads ----
                # separate tiles per kh block / XT half so the tile
                # dependency tracker lets kh=0 chunk-0 products start as
                # soon as XTa + A0 land (deps are whole-tile).
                # XTa = slots [0,48) for chunk 0; XTb = slots [16,64)
                # (overlapping) for chunk 1.
                XTa = xtpool.tile([128, 48 * 128], f16, tag="xta")
                nc.sync.dma_start(XTa[:], xt_in[d, :, :48 * 128])
                Akh = []
                for kh in range(3):
                    Ak = apool.tile([128, ABLK], f16, tag=f"a{kh}")
                    src = y_in[:].copy()
                    src.ap = VecI64Pair([[ABLK, 128], [1, ABLK]])
                    src.offset = (d * WPAD + kh) * ABLK
                    nc.sync.dma_start(Ak[:], src)
                    Akh.append(Ak)
                XTb = xtpool.tile([128, 48 * 128], f16, tag="xtb")
                nc.sync.dma_start(XTb[:], xt_in[d, :, 16 * 128:])

                # ---- 2 chunks of 16 c'-units ----
                for ch in range(2):
                    c0 = 16 * ch
                    P = prpool.tile([128, 16 * 1152], f16, tag="p")
                    # products: 4 groups of 4 units x 3 kh, one TT each
                    # (TensorTensor ISA mem patterns allow at most 3 free
                    # dims, so the kh axis gets its own instruction)
                    XTc = XTa if ch == 0 else XTb
                    sbase = 0 if ch == 0 else 16
                    for kh in range(3):
                        for gi in range(4):
                            kk = 4 * ch + gi        # global group id 0..7
                            cp0 = 4 * kk            # first c' of group
                            s0 = 4 * kk             # first XT slot of group
                            in0 = apv(Akh[kh][:], cp0 * WPAD,
                                      [[WPAD, 4], [1, 3], [1, 128]])
                            in1 = apv(XTc[:], (s0 - sbase + 3 * kh) * 128,
                                      [[1152, 4], [1, 384]])
                            po = apv(P[:], gi * 4608 + kh * 384,
                                     [[1152, 4], [1, 384]])
                            nc.vector.tensor_tensor(po, in0, in1, mult)
                    # tree over c2 (innermost 32), g = 16*36 = 576 groups
                    # emitted in halves so PE group h=0 starts while DVE
                    # computes the second half
                    g = 576
                    T1h = []
                    for hh in range(2):
                        off = hh * (g // 2)
                        Th = trpool.tile([128, g * 8], f16, tag=f"t1{hh}")
                        nc.vector.tensor_tensor(
                            apv(Th[:], 0, [[1, g * 8]]),
                            apv(P[:], off * 32, [[32, g // 2], [1, 16]]),
                            apv(P[:], off * 32 + 16,
                                [[32, g // 2], [1, 16]]), add)
                        T1h.append(Th)
                    # Remaining 16-way sum on the PE: identity-stationary
                    # matmuls accumulating the 16 strided T1 views into a
                    # PSUM fp32 bank (2 groups of 288 columns per chunk),
                    # then lrelu straight off PSUM (ACT) + one DVE add.
                    # the DVE final adds for the PREVIOUS chunk are only
                    # emitted now, so PE/ACT have a whole chunk of slack
                    # before DVE needs their results (no DVE stall).
                    flush_pending(keep=2)
                    for h in range(2):
                        PS = pspool.tile([128, 288], f32, tag="ps")
                        for j in range(16):
                            nc.tensor.matmul(
                                PS[:], ID[:],
                                apv(T1h[h][:], j, [[16, 288]]),
                                start=(j == 0), stop=(j == 15))
                        AB = abpool.tile([128, 288], f16, tag=f"ab{h}")
                        CC = abpool.tile([128, 288], f16, tag=f"cc{h}")
                        nc.scalar.activation(
                            AB[:], PS[:], mybir.ActivationFunctionType.Abs,
                            scale=0.4)
                        nc.scalar.activation(
                            CC[:], PS[:], mybir.ActivationFunctionType.Copy,
                            scale=0.6)
                        pending.append(
                            (AB, CC,
                             out[d, :, c0 * 36 + 288 * h:
                                 c0 * 36 + 288 * h + 288]))

            flush_pending()
    nc.finalize()
    return nc


def _get_program():
    if "nc" not in _PROG_CACHE:
        _PROG_CACHE["nc"] = _build_program()
    return _PROG_CACHE["nc"]


def _pack_xt(x):  # x [1,32,32,128,128] f32 -> [32, 128, NSLOT*128] fp16
    B, C_, D, H_, W_ = x.shape
    xt = np.zeros((D, 128, NSLOT, 128), np.float32)
    xd = x[0]  # [C, D, H, W]
    s = np.arange(NSLOT)
    m = s % 32
    for t in range(4):
        v = xd[:, :, 4 * m + t, :].reshape(C_, D, NSLOT, 32, 4)  # c d s wb j
        xt[:, 32 * t:32 * t + 32, :, :] = (
            v.transpose(1, 3, 2, 4, 0).reshape(D, 32, NSLOT, 128))
    return np.ascontiguousarray(xt.reshape(D, 128, NSLOT * 128)
                                ).astype(np.float16)


def kernel(x: np.ndarray, y: np.ndarray) -> np.ndarray:
    from concourse.bass_utils import run_bass_kernel_spmd

    x = np.ascontiguousarray(np.asarray(x, dtype=np.float32))
    y = np.ascontiguousarray(np.asarray(y, dtype=np.float32))
    B, C_, D, H_, W_ = x.shape
    assert (B, C_, D, H_, W_) == (1, 32, 32, 128, 128)

    # host prep: depth-shifted, H/W-padded y (fp16); packed XT slabs
    y_sp = np.zeros((D, WPAD, C_, WPAD), np.float16)
    y_sp[1:, 1:129, :, 1:129] = y[0].transpose(1, 2, 0, 3)[:-1].astype(
        np.float16)
    xt = _pack_xt(x)

    nc = _get_program()
    ident = np.eye(128, dtype=np.float16)
    in_maps = [
        {"xtin": xt[4 * j:4 * j + 4], "yin": y_sp[4 * j:4 * j + 4],
         "ident": ident}
        for j in range(N_CORES)
    ]
    res = run_bass_kernel_spmd(nc, in_maps, core_ids=list(range(N_CORES)),
                               trace=_RUN_OPTS["trace"])
    _LAST_RESULT["res"] = res
    packed = np.concatenate(
        [np.asarray(res.results[j]["out"], np.float32)
         for j in range(N_CORES)], axis=0)  # [32, 128, 1152]

    # host unpermute: [d, p, c'*36 + k*4 + j] -> [1, 9, D, H, W]
    a = packed.reshape(D, 4, 32, 32, 9, 4)                 # d t wb c' k j
    a = a.transpose(3, 4, 0, 1, 2, 5)                      # c' k d t wb j
    a = np.ascontiguousarray(a).reshape(9, 32, D, 4, 32, 4)  # k2 m d t wb j
    a = a.transpose(0, 2, 1, 3, 4, 5)                      # k2 d m t wb j
    a = np.ascontiguousarray(a).reshape(9, D, 128, 128)
    return a[None].astype(np.float32)
